# revision 40
# baseline (speedup 1.0000x reference)
"""Trainium2 Bass kernel for nn_MoETransformerBlock (MoE-LoRA ViT block).

Strategy: data-parallel over batch across 8 NeuronCores (2 batch elems per
core), weights replicated. No collectives. All activations are kept
feature-major [feature, token] in SBUF; LayerNorm gamma/beta are folded into
the following matmul weights on the host. The fat matmuls (QKV, V, proj,
fc1, fc2, LoRA, adapters) run in fp8e4m3 DoubleRow mode (K=256 per
instruction, 2x bf16 throughput); weights are pre-scaled by 256 on the host
and dequantized on PSUM eviction. Attention computes transposed scores
s_T[k, q] per head so softmax denominators come for free from a
ones-augmented V matrix.
"""

import sys

sys.path.insert(0, "/opt/trn_rl_repo")

import numpy as np
import ml_dtypes

BF16 = ml_dtypes.bfloat16
F8 = ml_dtypes.float8_e4m3

# ---- problem constants (hardcoded; must match reference.py) ----
B, N, E, H, HD = 16, 577, 1024, 16, 64
LORA_E, LORA_R = 4, 16
AD_E, AD_D = 4, 64
FF = 4 * E
NCORES = 8
BLOC = B // NCORES        # 2 batch elems per core
T = BLOC * N              # 1154 tokens per core
NKT = E // 128            # 8 feature k-tiles
NQT = 5                   # token tiles per batch: 4x128 + 65
TP = 1168                 # fp8 lhsT tiles pad the token stride to 16B multiples

SW = 256.0                # fp8 weight pre-scale
SI = 1.0 / SW             # dequant scale on eviction
VS = 16.0                 # extra V/ctx pre-scale (lifts ctx out of subnormals)

# per-batch token tiles (global token start, size)
TOKTILES = [(b * N + i * 128, min(128, N - i * 128))
            for b in range(BLOC) for i in range(NQT)]
# free-dim chunks (global token start, size) for batch-agnostic ops
CHUNKS = [(0, 512), (512, 512), (1024, 130)]
QCHUNKS = [(0, 512), (512, 65)]  # batch-local (attention)


def _build(tc, io, cfg):
    """Emit the Tile program. io: dict name -> bass.AP (dram)."""
    import concourse.bass as bass
    import concourse.mybir as mybir
    from concourse.masks import make_identity

    nc = tc.nc
    f32 = mybir.dt.float32
    bf = mybir.dt.bfloat16
    f8 = mybir.dt.float8e4
    AF = mybir.ActivationFunctionType
    OP = mybir.AluOpType
    DR = mybir.MatmulPerfMode.DoubleRow

    def mm(out, lhsT, rhs, start, stop):
        nc.tensor.matmul(out, lhsT, rhs, start=start, stop=stop)

    def mmdr(out, lhsT, rhs, start, stop):
        nc.tensor.matmul(out, lhsT, rhs, start=start, stop=stop, perf_mode=DR)

    NDR = NKT // 2  # 4 DoubleRow k-pairs over the embed dim

    import contextlib
    ctx = contextlib.ExitStack()
    with ctx:
        sp = ctx.enter_context(tc.tile_pool(name="persist", bufs=1))

        # ---------- persistent SBUF ----------
        x_sb = sp.tile([128, NKT, T], bf)          # tokens -> t1 residual
        x_fm_r = io["x_fm"].rearrange("(k p) t -> p k t", p=128)
        for (cs, cn) in CHUNKS:
            nc.sync.dma_start(x_sb[:, :, cs:cs + cn], x_fm_r[:, :, cs:cs + cn])

        ident = sp.tile([128, 128], f32)
        make_identity(nc, ident)
        identb = sp.tile([128, 128], bf)
        make_identity(nc, identb)
        ones_c = sp.tile([128, 1], bf)             # column of ones (lhsT for colsum)
        nc.vector.memset(ones_c, 1.0)
        ones_p8 = sp.tile([128, 2, 16], f8)        # DoubleRow ones pair (16B step)
        nc.vector.memset(ones_p8, 1.0)
        eps_t = sp.tile([1, 1], f32)
        nc.vector.memset(eps_t, 1e-6)

        # small per-partition bias tiles
        if cfg["has_bqk"]:
            bqk_sb = sp.tile([128, 16], f32)
            nc.sync.dma_start(bqk_sb, io["bqk"].rearrange("(m p) -> p m", p=128))
        bh_sb = sp.tile([64, 1], f32)
        nc.sync.dma_start(bh_sb, io["bh"].rearrange("(p o) -> p o", o=1))
        bfc1_sb = sp.tile([128, 32], f32)
        nc.sync.dma_start(bfc1_sb, io["bfc1"].rearrange("(m p) -> p m", p=128))
        bfc2_sb = sp.tile([128, 8], f32)
        nc.sync.dma_start(bfc2_sb, io["bfc2"].rearrange("(m p) -> p m", p=128))
        bad_sb = sp.tile([128, 2], f32)
        nc.sync.dma_start(bad_sb, io["bad"].rearrange("(m p) -> p m", p=128))
        elora_sb = sp.tile([4, 64], f8)
        nc.sync.dma_start(elora_sb, io["elora"])
        ead_sb = sp.tile([4, 256], bf)
        nc.sync.dma_start(ead_sb, io["ead"])
        if cfg["has_vbias"]:
            bv_sb = sp.tile([1, E], bf)
            nc.sync.dma_start(bv_sb, io["bv"].rearrange("(o e) -> o e", o=1))
            ones_r = sp.tile([1, 128], bf)         # row of ones (lhsT)
            nc.vector.memset(ones_r, 1.0)

        # ctx_sb outlives the other attention buffers (consumed by proj)
        cctx = contextlib.ExitStack()
        cp_ = cctx.enter_context(tc.tile_pool(name="ctx_buf", bufs=1))
        ctx_sb = cp_.tile([128, NKT, T], f8)       # attention out, feature-major

        # attention-lifetime buffers: freed (actx.close()) before the MLP phase
        actx = contextlib.ExitStack()
        ap_ = actx.enter_context(tc.tile_pool(name="attn_bufs", bufs=1))
        qk_sb = ap_.tile([128, 16, T], f8)         # q (mt 0..7), k (mt 8..15)
        # token-major v, 66 cols/head: 64 v + ones (denominator) + zero pad
        v_sb = ap_.tile([128, 2 * NQT, H * 66], f8)
        nc.vector.memset(
            v_sb.rearrange("p t (h c) -> p t h c", c=66)[:, :, :, 64:65], 1.0)
        nc.vector.memset(
            v_sb.rearrange("p t (h c) -> p t h c", c=66)[:, :, :, 65:66], 0.0)
        # attention out, token-major [q-token, head, dim] per (b, qsub)
        ctok = ap_.tile([128, 2 * NQT, H, 64], bf)

        # ---------- helpers ----------
        def layer_norm(src, dst, pool, lnp, pp):
            """Feature-major LN: dst[f,t] = (src[f,t]-mean_t)*rstd_t.
            Per-chunk colsums feed one batched row-op pass (slots: 0 sum,
            1 sumsq, 2 mean, 3 m2, 4 var, 5 ln(var), 6 rstd)."""
            rows = lnp.tile([1, 3, 4, 512], f32, tag="ln_rows")
            browz = lnp.tile([1, 3, 2, 512], bf, tag="ln_brows")
            nc.vector.memset(rows[:, 2, 0:2, 130:512], 0.0)
            for ci, (cs, cn) in enumerate(CHUNKS):
                sum_ps = pp.tile([1, 512], f32, tag="ln_sum")
                sq_ps = pp.tile([1, 512], f32, tag="ln_sq")
                for kt in range(NKT):
                    mm(sum_ps[:, :cn], ones_c, src[:, kt, cs:cs + cn],
                       start=(kt == 0), stop=(kt == NKT - 1))
                for kd in range(NDR):
                    xsq = pool.tile([128, 2, 512], f8, tag="ln_xsq")
                    nc.scalar.activation(xsq[:, 0, :cn],
                                         src[:, 2 * kd, cs:cs + cn], AF.Square)
                    nc.scalar.activation(xsq[:, 1, :cn],
                                         src[:, 2 * kd + 1, cs:cs + cn], AF.Square)
                    mmdr(sq_ps[:, :cn], ones_p8[:, :, 0:1], xsq[:, :, :cn],
                         start=(kd == 0), stop=(kd == NDR - 1))
                nc.vector.tensor_copy(rows[:, ci, 0, :cn], sum_ps[:, :cn])
                nc.vector.tensor_copy(rows[:, ci, 1, :cn], sq_ps[:, :cn])
            # slot reuse: 0 sum->m2->rstd, 1 sq->lnv, 2 mean, 3 var
            mean = rows[:, :, 2, :]
            var = rows[:, :, 3, :]
            nc.vector.tensor_scalar_mul(mean, rows[:, :, 0, :], 1.0 / E)
            nc.vector.tensor_mul(rows[:, :, 0, :], mean, mean)
            nc.vector.scalar_tensor_tensor(var, rows[:, :, 1, :], 1.0 / E,
                                           rows[:, :, 0, :],
                                           op0=OP.mult, op1=OP.subtract)
            # rstd = exp(-0.5 * ln(var + eps)) -- stays in the exp ACT table
            nc.scalar.activation(rows[:, :, 1, :], var, AF.Ln, bias=eps_t)
            nc.scalar.activation(rows[:, :, 0, :], rows[:, :, 1, :],
                                 AF.Exp, scale=-0.5)
            nc.vector.tensor_copy(browz[:, :, 0, :], rows[:, :, 0, :])
            nc.vector.tensor_mul(browz[:, :, 1, :], mean, rows[:, :, 0, :])
            for ci, (cs, cn) in enumerate(CHUNKS):
                R_bc = pool.tile([128, 512], bf, tag="ln_Rbc")
                MR_bc = pool.tile([128, 512], bf, tag="ln_MRbc")
                nc.gpsimd.partition_broadcast(R_bc[:, :cn], browz[:, ci, 0, :cn])
                nc.gpsimd.partition_broadcast(MR_bc[:, :cn], browz[:, ci, 1, :cn])
                for kt in range(NKT):
                    tmp = pool.tile([128, 512], bf, tag="ln_tmp")
                    nc.vector.tensor_mul(tmp[:, :cn], src[:, kt, cs:cs + cn],
                                         R_bc[:, :cn])
                    nc.vector.tensor_sub(dst[:, kt, cs:cs + cn], tmp[:, :cn],
                                         MR_bc[:, :cn])

        def gates(src, w_sb, bias_bc, dst, pool, pp, zscale, fp8):
            """softmax over 4 experts -> dst [4, T] feature-major."""
            for (ts, tn) in TOKTILES:
                z_ps = pp.tile([128, 4], f32, tag="gz")
                if fp8:
                    for kd in range(NDR):
                        mmdr(z_ps[:tn], src[:, 2 * kd:2 * kd + 2, ts:ts + tn],
                             w_sb[:, 2 * kd:2 * kd + 2, :],
                             start=(kd == 0), stop=(kd == NDR - 1))
                else:
                    for kt in range(NKT):
                        mm(z_ps[:tn], src[:, kt, ts:ts + tn], w_sb[:, kt, :],
                           start=(kt == 0), stop=(kt == NKT - 1))
                ex = pool.tile([128, 4], f32, tag="g_ex")
                den = pool.tile([128, 1], f32, tag="g_den")
                if bias_bc is not None:
                    zt = pool.tile([128, 4], f32, tag="g_zt")
                    nc.vector.scalar_tensor_tensor(
                        zt[:tn], z_ps[:tn], zscale, bias_bc[:tn],
                        op0=OP.mult, op1=OP.add)
                    nc.scalar.activation(ex[:tn], zt[:tn], AF.Exp,
                                         accum_out=den[:tn])
                else:
                    # logits are small (std ~0.6); exp straight off PSUM
                    nc.scalar.activation(ex[:tn], z_ps[:tn], AF.Exp,
                                         scale=zscale, accum_out=den[:tn])
                rr = pool.tile([128, 1], f32, tag="g_rr")
                nc.vector.reciprocal_approx_fast(rr[:tn], den[:tn])
                gt = pool.tile([128, 4], f32, tag="g_gt")
                nc.vector.tensor_scalar_mul(gt[:tn], ex[:tn], rr[:tn])
                tp = pp.tile([4, 128], f32, tag="g_tp")
                nc.tensor.transpose(tp[:, :tn], gt[:tn, :], ident[:tn, :tn])
                nc.scalar.copy(dst[:, ts:ts + tn], tp[:, :tn])

        # ========== phase 1: LN1 + gates + lora h + v + (qk || attention) ==========
        with tc.tile_pool(name="p1", bufs=3) as p1, \
             tc.tile_pool(name="p1w", bufs=1) as p1w, \
             tc.tile_pool(name="lnp1", bufs=1) as lnp1:
            n1 = p1w.tile([128, NKT, TP], f8)
            g_lora = p1w.tile([4, T], f8)
            h_lora = p1w.tile([64, T], f8)
            gh_lora = p1w.tile([64, T], f8)
            wgl_sb = p1w.tile([128, NKT, 4], f8)
            nc.sync.dma_start(wgl_sb, io["wgl"].rearrange("(k p) c -> p k c", p=128))
            at_sb = p1w.tile([128, NKT, 64], f8)
            nc.sync.dma_start(at_sb, io["at"].rearrange("(k p) c -> p k c", p=128))
            bgl_bc = None
            if cfg["has_bgl"]:
                bgl_bc = p1w.tile([128, 4], f32)
                nc.sync.dma_start(bgl_bc, io["bgl"].to_broadcast((128, 4)))
            wqk_sb = p1w.tile([128, NKT, 2048], f8)
            nc.sync.dma_start(wqk_sb, io["wqk"].rearrange("(k p) m -> p k m", p=128))
            bqkvT_sb = p1w.tile([64, 3 * E], f8)
            nc.sync.dma_start(bqkvT_sb, io["bqkvT"])

            with tc.tile_pool(name="ppLN", bufs=2, space="PSUM") as ppLN:
                layer_norm(x_sb, n1, p1, lnp1, ppLN)
            with tc.tile_pool(name="ppG", bufs=2, space="PSUM") as ppG:
                gates(n1, wgl_sb, bgl_bc, g_lora, p1, ppG, SI, True)
                for (cs, cn) in CHUNKS:
                    h_ps = ppG.tile([64, 512], f32, tag="h_ps")
                    for kd in range(NDR):
                        mmdr(h_ps[:, :cn], at_sb[:, 2 * kd:2 * kd + 2, :],
                             n1[:, 2 * kd:2 * kd + 2, cs:cs + cn],
                             start=(kd == 0), stop=(kd == NDR - 1))
                    nc.scalar.activation(h_lora[:, cs:cs + cn], h_ps[:, :cn],
                                         AF.Identity, bias=bh_sb, scale=SI)
                    ge_ps = ppG.tile([64, 512], f32, tag="ge_ps")
                    mm(ge_ps[:, :cn], elora_sb, g_lora[:, cs:cs + cn],
                       start=True, stop=True)
                    nc.vector.tensor_mul(gh_lora[:, cs:cs + cn],
                                         h_lora[:, cs:cs + cn], ge_ps[:, :cn])

            # ---------------- v (token-major, ones-interleaved) ----------------
            with tc.tile_pool(name="wvp", bufs=1) as wvp, \
                 tc.tile_pool(name="pp_v", bufs=6, space="PSUM") as pp_v:
                wv_sb = wvp.tile([128, NKT, E], f8)
                nc.sync.dma_start(wv_sb,
                                  io["wv"].rearrange("(k p) m -> p k m", p=128))
                for it, (ts, tn) in enumerate(TOKTILES):
                    for oc in (0, 512):
                        ps = pp_v.tile([128, 512], f32, tag="v_ps")
                        for kd in range(NDR):
                            mmdr(ps[:tn], n1[:, 2 * kd:2 * kd + 2, ts:ts + tn],
                                 wv_sb[:, 2 * kd:2 * kd + 2, oc:oc + 512],
                                 start=(kd == 0), stop=False)
                        mm(ps[:tn], gh_lora[:, ts:ts + tn],
                           bqkvT_sb[:, 2048 + oc:2048 + oc + 512],
                           start=False, stop=not cfg["has_vbias"])
                        if cfg["has_vbias"]:
                            mm(ps[:tn], ones_r[:, :tn], bv_sb[:, oc:oc + 512],
                               start=False, stop=True)
                        dst = v_sb[:tn, it, :].rearrange("p (h c) -> p h c", c=66)[
                            :, oc // 64:oc // 64 + 8, 0:64]
                        src = ps[:tn, :].rearrange("p (h c) -> p h c", c=64)
                        nc.vector.tensor_scalar_mul(dst, src, SI * VS)

            # -------- interleaved qk Mtiles + attention (token-major ctx) -----
            with tc.tile_pool(name="pe_", bufs=8) as pe_, \
                 tc.tile_pool(name="psm", bufs=6) as psm, \
                 tc.tile_pool(name="pp_qk", bufs=1, space="PSUM") as pp_qk, \
                 tc.tile_pool(name="pp_s", bufs=2, space="PSUM") as pp_s, \
                 tc.tile_pool(name="pp_s4", bufs=1, space="PSUM") as pp_s4, \
                 tc.tile_pool(name="pp_cx", bufs=2, space="PSUM") as pp_cx:

                def qk_mt(mt):
                    for (cs, cn) in CHUNKS:
                        ps = pp_qk.tile([128, 512], f32, tag="qk_ps")
                        for kd in range(NDR):
                            mmdr(ps[:, :cn],
                                 wqk_sb[:, 2 * kd:2 * kd + 2,
                                        mt * 128:(mt + 1) * 128],
                                 n1[:, 2 * kd:2 * kd + 2, cs:cs + cn],
                                 start=(kd == 0), stop=False)
                        mm(ps[:, :cn], bqkvT_sb[:, mt * 128:(mt + 1) * 128],
                           gh_lora[:, cs:cs + cn], start=False, stop=True)
                        if cfg["has_bqk"]:
                            nc.scalar.activation(qk_sb[:, mt, cs:cs + cn],
                                                 ps[:, :cn], AF.Identity,
                                                 bias=bqk_sb[:, mt:mt + 1],
                                                 scale=SI)
                        else:
                            nc.vector.tensor_scalar_mul(qk_sb[:, mt, cs:cs + cn],
                                                        ps[:, :cn], SI)

                def attn_head(h, b, qs, qn):
                    """exp tiles for head h over q-window, kt-paired for DR."""
                    mtq, mtk = h // 2, 8 + h // 2
                    fo = 64 * (h % 2)
                    g0 = b * N + qs
                    q_sl = qk_sb[fo:fo + 64, mtq, g0:g0 + qn]
                    es = []
                    for pair in range(2):
                        sp_ = pp_s.tile([128, 2, 512], f32, tag="s_pair")
                        for i in range(2):
                            kt = 2 * pair + i
                            ks = b * N + kt * 128
                            mm(sp_[:, i, :qn], qk_sb[fo:fo + 64, mtk, ks:ks + 128],
                               q_sl, start=True, stop=True)
                        ep = pe_.tile([128, 2, 512], f8, tag=f"e{pair}")
                        nc.scalar.activation(ep[:, :, :qn], sp_[:, :, :qn],
                                             AF.Exp, scale=HD ** -0.5)
                        es.append(ep)
                    s4 = pp_s4.tile([65, 512], f32, tag="s_tail")
                    ks = b * N + 512
                    mm(s4[:, :qn], qk_sb[fo:fo + 64, mtk, ks:ks + 65],
                       q_sl, start=True, stop=True)
                    e4 = pe_.tile([65, 512], f8, tag="e4")
                    nc.scalar.activation(e4[:, :qn], s4[:, :qn],
                                         AF.Exp, scale=HD ** -0.5)
                    return es[0], es[1], e4

                def attn_ctx(hp, b, qs, qn, eset0, eset1):
                    """ctx for head pair hp over 128-token q-subtiles."""
                    h0 = 2 * hp
                    for sub in range((qn + 127) // 128):
                        ss = sub * 128
                        sn = min(128, qn - ss)
                        ti = b * NQT + (qs + ss) // 128
                        cx2 = pp_cx.tile([128, 2, 66], f32, tag="cx2")
                        for hi, (e01, e23, e4) in enumerate((eset0, eset1)):
                            hh = h0 + hi
                            mmdr(cx2[:sn, hi, :], e01[:, :, ss:ss + sn],
                                 v_sb[:, b * NQT + 0:b * NQT + 2,
                                      hh * 66:hh * 66 + 66],
                                 start=True, stop=False)
                            mmdr(cx2[:sn, hi, :], e23[:, :, ss:ss + sn],
                                 v_sb[:, b * NQT + 2:b * NQT + 4,
                                      hh * 66:hh * 66 + 66],
                                 start=False, stop=False)
                            mm(cx2[:sn, hi, :], e4[:, ss:ss + sn],
                               v_sb[0:65, b * NQT + 4, hh * 66:hh * 66 + 66],
                               start=False, stop=True)
                        dn2 = psm.tile([128, 2], f32, tag="dn2")
                        nc.vector.tensor_copy(
                            dn2[:sn], cx2[:sn, :, 64:65].rearrange(
                                "p a o -> p (a o)"))
                        r2 = psm.tile([128, 2], f32, tag="r2")
                        nc.vector.reciprocal_approx_fast(r2[:sn], dn2[:sn])
                        for hi in range(2):
                            nc.vector.tensor_scalar_mul(
                                ctok[:sn, ti, h0 + hi, :], cx2[:sn, hi, 0:64],
                                r2[:sn, hi:hi + 1])

                for j in range(H // 2):
                    qk_mt(j)
                    qk_mt(8 + j)
                    for b in range(BLOC):
                        for (qs, qn) in QCHUNKS:
                            e_h0 = attn_head(2 * j, b, qs, qn)
                            e_h1 = attn_head(2 * j + 1, b, qs, qn)
                            attn_ctx(j, b, qs, qn, e_h0, e_h1)

            # -------- transpose ctx back to feature-major for proj --------
            with tc.tile_pool(name="pp_tp", bufs=4, space="PSUM") as pp_tp:
                for ti in range(2 * NQT):
                    b, qt = ti // NQT, ti % NQT
                    g0 = b * N + qt * 128
                    tn = min(128, N - qt * 128)
                    for mt in range(NKT):
                        tp = pp_tp.tile([128, 128], bf, tag="tp")
                        nc.tensor.transpose(
                            tp[:, :tn],
                            ctok[:tn, ti, 2 * mt:2 * mt + 2, :].rearrange(
                                "p a c -> p (a c)"),
                            identb[:tn, :tn])
                        nc.vector.tensor_copy(ctx_sb[:, mt, g0:g0 + tn],
                                              tp[:, :tn])

        # free attention-lifetime buffers (ctx_sb stays for proj)
        actx.close()

        # ================= phase 3: LN2 + MLP + adapter =================
        # Split MLP: fc2@gelu(fc1@n2) = M@n2 + W2@t, with M = 0.5*W2@W1
        # (bf16, exact linear part) and t = gelu(x) - 0.5x (small; fp8).
        # fc1 runs bf16; fc2-on-t runs fp8 DoubleRow with W2n = -W2 so the
        # Pool engine can produce t' = 0.5*ps - gelu in one pass.
        with tc.tile_pool(name="p3", bufs=2) as p3, \
             tc.tile_pool(name="p3w", bufs=1) as p3w, \
             tc.tile_pool(name="lnp3", bufs=1) as lnp3:
            n2 = p3w.tile([128, NKT, T], bf, tag="n2")
            g_ad = p3w.tile([4, T], bf)
            wgad_sb = p3w.tile([128, NKT, 4], bf)
            nc.sync.dma_start(wgad_sb, io["wgad"].rearrange("(k p) c -> p k c", p=128))
            bgad_bc = None
            if cfg["has_bgad"]:
                bgad_bc = p3w.tile([128, 4], f32)
                nc.sync.dma_start(bgad_bc, io["bgad"].to_broadcast((128, 4)))

            # proj + residual (t1 overwrites x_sb), chunk-outer so LN2's
            # colsums for chunk c start while proj works on chunk c+1
            with tc.tile_pool(name="wpp", bufs=1) as wpp, \
                 tc.tile_pool(name="pp_pr", bufs=2, space="PSUM") as pp_pr, \
                 tc.tile_pool(name="ppLN2", bufs=2, space="PSUM") as ppLN2:
                wp_sb = wpp.tile([128, NKT, E], f8)
                nc.sync.dma_start(wp_sb,
                                  io["wp"].rearrange("(k p) m -> p k m", p=128))
                if cfg["has_bp"]:
                    bp_sb = wpp.tile([128, 8], f32)
                    nc.sync.dma_start(bp_sb,
                                      io["bp"].rearrange("(m p) -> p m", p=128))
                for (cs, cn) in CHUNKS:
                    for mt in range(NKT):
                        ps = pp_pr.tile([128, 512], f32, tag="pr_ps")
                        for kd in range(NDR):
                            mmdr(ps[:, :cn],
                                 wp_sb[:, 2 * kd:2 * kd + 2,
                                       mt * 128:(mt + 1) * 128],
                                 ctx_sb[:, 2 * kd:2 * kd + 2, cs:cs + cn],
                                 start=(kd == 0), stop=(kd == NDR - 1))
                        if cfg["has_bp"]:
                            tmp = wpp.tile([128, 512], f32, tag="prtmp")
                            nc.vector.tensor_scalar(
                                tmp[:, :cn], ps[:, :cn], SI / VS,
                                bp_sb[:, mt:mt + 1], op0=OP.mult, op1=OP.add)
                            nc.vector.tensor_add(
                                x_sb[:, mt, cs:cs + cn], tmp[:, :cn],
                                x_sb[:, mt, cs:cs + cn])
                        else:
                            nc.vector.scalar_tensor_tensor(
                                x_sb[:, mt, cs:cs + cn], ps[:, :cn], SI / VS,
                                x_sb[:, mt, cs:cs + cn],
                                op0=OP.mult, op1=OP.add)
                layer_norm(x_sb, n2, p3, lnp3, ppLN2)
            with tc.tile_pool(name="ppG2", bufs=2, space="PSUM") as ppG2:
                gates(n2, wgad_sb, bgad_bc, g_ad, p3, ppG2, 1.0, False)

            wad_sb = p3w.tile([128, NKT, 256], bf)
            nc.sync.dma_start(wad_sb, io["wad"].rearrange("(k p) c -> p k c", p=128))
            up_sb = p3w.tile([128, 2, E], bf)
            nc.sync.dma_start(
                up_sb, io["upaug"][0:256, :].rearrange("(k p) e -> p k e", p=128))
            up_tail = p3w.tile([4, E], bf)
            nc.sync.dma_start(up_tail, io["upaug"][256:260, :])
            m_sb = p3w.tile([128, NKT, E], bf)
            nc.sync.dma_start(m_sb, io["mlin"].rearrange("(k p) m -> p k m", p=128))
            partial = p3w.tile([128, NKT, T], bf)   # fc2 half-0 partial sums

            wfc1_all = io["wfc1"].rearrange("(k p) m -> p k m", p=128)
            wfc2_all = io["wfc2"].rearrange("(k p) m -> p k m", p=128)
            FH = FF // 2 // 128   # 16 fc1-Mtiles (= fc2-ktiles) per half
            FDR = FH // 2         # 8 DoubleRow pairs per half

            with tc.tile_pool(name="p3s", bufs=1) as p3s, \
                 tc.tile_pool(name="p3c", bufs=1) as p3c, \
                 tc.tile_pool(name="p3t", bufs=2) as p3t, \
                 tc.tile_pool(name="pp_f1", bufs=4, space="PSUM") as pp_f1, \
                 tc.tile_pool(name="pp_f2", bufs=4, space="PSUM") as pp_f2:
                for ffh in range(2):
                    wfc1_h = p3s.tile([128, NKT, FH * 128], bf, tag="wfc1h")
                    nc.sync.dma_start(
                        wfc1_h, wfc1_all[:, :, ffh * FH * 128:(ffh + 1) * FH * 128])
                    wfc2_h = p3s.tile([128, FH, E], f8, tag="wfc2h")
                    nc.sync.dma_start(
                        wfc2_h, wfc2_all[:, ffh * FH:(ffh + 1) * FH, :])
                    for ci, (cs, cn) in enumerate(CHUNKS):
                        if ffh == 1:
                            # adapter: gated gelu bottleneck (second half only)
                            gah = p3t.tile([128, 2, 512], bf, tag="gah")
                            for amt in range(2):
                                ps = pp_f1.tile([128, 512], f32, tag="f1_ps")
                                for kt in range(NKT):
                                    mm(ps[:, :cn],
                                       wad_sb[:, kt, amt * 128:(amt + 1) * 128],
                                       n2[:, kt, cs:cs + cn],
                                       start=(kt == 0), stop=(kt == NKT - 1))
                                ah = p3t.tile([128, 512], bf, tag="ah")
                                nc.scalar.activation(ah[:, :cn], ps[:, :cn],
                                                     AF.Gelu,
                                                     bias=bad_sb[:, amt:amt + 1])
                                ge = pp_f2.tile([128, 512], f32, tag="f2_ps")
                                mm(ge[:, :cn],
                                   ead_sb[:, amt * 128:(amt + 1) * 128],
                                   g_ad[:, cs:cs + cn], start=True, stop=True)
                                nc.vector.tensor_mul(gah[:, amt, :cn], ah[:, :cn],
                                                     ge[:, :cn])
                        # fc1 (bf16) -> gelu -> t' = 0.5*ps - gelu (fp8)
                        t8 = p3c.tile([128, FH, 512], f8, tag="t8")
                        for mt in range(FH):
                            ps = pp_f1.tile([128, 512], f32, tag="f1_ps")
                            for kt in range(NKT):
                                mm(ps[:, :cn],
                                   wfc1_h[:, kt, mt * 128:(mt + 1) * 128],
                                   n2[:, kt, cs:cs + cn],
                                   start=(kt == 0), stop=(kt == NKT - 1))
                            gt = p3t.tile([128, 512], bf, tag="gt")
                            nc.scalar.activation(
                                gt[:, :cn], ps[:, :cn], AF.Gelu,
                                bias=bfc1_sb[:, ffh * FH + mt:ffh * FH + mt + 1])
                            nc.vector.scalar_tensor_tensor(
                                t8[:, mt, :cn], ps[:, :cn], 0.5, gt[:, :cn],
                                op0=OP.mult, op1=OP.subtract)
                        # fc2 half on t (+ M@n2 in half 0, adapter-up in half 1)
                        for mt in range(NKT):
                            ps = pp_f2.tile([128, 512], f32, tag="f2_ps")
                            if ffh == 0:
                                for kt in range(NKT):
                                    mm(ps[:, :cn],
                                       m_sb[:, kt, mt * 128:(mt + 1) * 128],
                                       n2[:, kt, cs:cs + cn],
                                       start=(kt == 0), stop=False)
                            else:
                                mm(ps[:, :cn],
                                   up_sb[:, 0, mt * 128:(mt + 1) * 128],
                                   gah[:, 0, :cn], start=True, stop=False)
                                mm(ps[:, :cn],
                                   up_sb[:, 1, mt * 128:(mt + 1) * 128],
                                   gah[:, 1, :cn], start=False, stop=False)
                                mm(ps[:, :cn], up_tail[:, mt * 128:(mt + 1) * 128],
                                   g_ad[:, cs:cs + cn], start=False, stop=False)
                            for kd in range(FDR):
                                mmdr(ps[:, :cn],
                                     wfc2_h[:, 2 * kd:2 * kd + 2,
                                            mt * 128:(mt + 1) * 128],
                                     t8[:, 2 * kd:2 * kd + 2, :cn],
                                     start=False, stop=(kd == FDR - 1))
                            if ffh == 0:
                                nc.vector.tensor_scalar(
                                    partial[:, mt, cs:cs + cn], ps[:, :cn],
                                    SI, bfc2_sb[:, mt:mt + 1],
                                    op0=OP.mult, op1=OP.add)
                            else:
                                ot = p3t.tile([128, 512], f32, tag="ot")
                                nc.vector.scalar_tensor_tensor(
                                    ot[:, :cn], ps[:, :cn], SI,
                                    partial[:, mt, cs:cs + cn],
                                    op0=OP.mult, op1=OP.add)
                                otb = p3t.tile([128, 512], bf, tag="otb")
                                nc.vector.tensor_add(otb[:, :cn], ot[:, :cn],
                                                     x_sb[:, mt, cs:cs + cn])
                                nc.sync.dma_start(
                                    io["out_fm"].rearrange(
                                        "(k p) t -> p k t", p=128)[
                                        :, mt, cs:cs + cn], otb[:, :cn])
        cctx.close()


def _prep_weights(inputs):
    """Host-side weight preparation (LN folding, transposes, fp8 casts)."""
    f = np.float32
    g1 = np.asarray(inputs["ln1_g"], f)
    b1 = np.asarray(inputs["ln1_b"], f)
    g2 = np.asarray(inputs["ln2_g"], f)
    b2 = np.asarray(inputs["ln2_b"], f)
    qkv_w = np.asarray(inputs["qkv_w"], f)
    Wq = qkv_w * g1[None, :]
    bqkv = np.asarray(inputs["qkv_b"], f) + qkv_w @ b1
    A = np.asarray(inputs["lora_A"], f)
    Afold = (A * g1[None, None, :]).reshape(LORA_E * LORA_R, E)
    Bm = np.asarray(inputs["lora_B"], f)
    lgw = np.asarray(inputs["lora_gate_w"], f)
    fc1_w = np.asarray(inputs["fc1_w"], f)
    fc2_w = np.asarray(inputs["fc2_w"], f)
    adg = np.asarray(inputs["ad_gate_w"], f)
    add_w = np.asarray(inputs["ad_down_w"], f).reshape(AD_E * AD_D, E)
    adu_w = np.asarray(inputs["ad_up_w"], f)

    elora = np.zeros((LORA_E, LORA_E * LORA_R), f)
    for x in range(LORA_E):
        elora[x, x * LORA_R:(x + 1) * LORA_R] = 1.0
    ead = np.zeros((AD_E, AD_E * AD_D), f)
    for x in range(AD_E):
        ead[x, x * AD_D:(x + 1) * AD_D] = 1.0

    def q8(a):
        return np.ascontiguousarray(np.asarray(a, f) * SW).astype(F8)

    bv = bqkv[2 * E:]
    bgl = lgw @ b1
    bgad = adg @ b2
    bp = np.asarray(inputs["proj_b"], f)
    w = {
        "wqk": q8(Wq[:2 * E].T),
        "wv": q8(Wq[2 * E:].T),
        "bqk": np.ascontiguousarray(bqkv[:2 * E]),
        "bv": bv.astype(BF16),
        "at": q8(Afold.T),
        "bh": (A.reshape(64, E) @ b1).astype(f),
        "bqkvT": q8(np.transpose(Bm, (0, 2, 1)).reshape(64, 3 * E)),
        "wgl": q8((lgw * g1[None, :]).T),
        "bgl": bgl.astype(f),
        "elora": elora.astype(F8),
        "wp": q8(np.asarray(inputs["proj_w"], f).T),
        "bp": bp,
        "wfc1": np.ascontiguousarray((fc1_w * g2[None, :]).T).astype(BF16),
        "bfc1": (np.asarray(inputs["fc1_b"], f) + fc1_w @ b2).astype(f),
        "wfc2": q8(-fc2_w.T),
        "bfc2": np.asarray(inputs["fc2_b"], f),
        "mlin": np.ascontiguousarray(
            ((0.5 * SW) * (fc2_w @ (fc1_w * g2[None, :]))).T).astype(BF16),
        "wgad": np.ascontiguousarray((adg * g2[None, :]).T).astype(BF16),
        "bgad": bgad.astype(f),
        "wad": np.ascontiguousarray((add_w * g2[None, :]).T).astype(BF16),
        "bad": (np.asarray(inputs["ad_down_b"], f).reshape(AD_E * AD_D)
                + add_w @ b2).astype(f),
        "upaug": np.ascontiguousarray(SW * np.concatenate(
            [np.transpose(adu_w, (0, 2, 1)).reshape(AD_E * AD_D, E),
             np.asarray(inputs["ad_up_b"], f)], axis=0)).astype(BF16),
        "ead": ead.astype(BF16),
    }
    cfg = {
        "has_vbias": bool(np.abs(bv).max() > 0),
        "has_bqk": bool(np.abs(bqkv[:2 * E]).max() > 0),
        "has_bgl": bool(np.abs(bgl).max() > 0),
        "has_bgad": bool(np.abs(bgad).max() > 0),
        "has_bp": bool(np.abs(bp).max() > 0),
    }
    return w, cfg


_CACHE = {}


def _get_program(cfg):
    key = tuple(sorted(cfg.items()))
    if key in _CACHE:
        return _CACHE[key]
    from concourse import bacc
    import concourse.tile as tile
    import concourse.mybir as mybir

    nc = bacc.Bacc("TRN2", target_bir_lowering=False, debug=False,
                   enable_asserts=False, num_devices=NCORES)
    f32 = mybir.dt.float32
    bf = mybir.dt.bfloat16
    f8 = mybir.dt.float8e4
    shapes = {
        "x_fm": ([E, T], bf),
        "wqk": ([E, 2 * E], f8), "wv": ([E, E], f8),
        "bqk": ([2 * E], f32), "bv": ([E], bf),
        "at": ([E, 64], f8), "bh": ([64], f32), "bqkvT": ([64, 3 * E], f8),
        "wgl": ([E, 4], f8), "bgl": ([4], f32),
        "elora": ([4, 64], f8), "ead": ([4, 256], bf),
        "wp": ([E, E], f8), "bp": ([E], f32),
        "wfc1": ([E, FF], bf), "bfc1": ([FF], f32),
        "wfc2": ([FF, E], f8), "bfc2": ([E], f32),
        "mlin": ([E, E], bf),
        "wgad": ([E, 4], bf), "bgad": ([4], f32),
        "wad": ([E, 256], bf), "bad": ([256], f32),
        "upaug": ([260, E], bf),
    }
    skip = set()
    if not cfg["has_bqk"]:
        skip.add("bqk")
    if not cfg["has_vbias"]:
        skip.add("bv")
    if not cfg["has_bgl"]:
        skip.add("bgl")
    if not cfg["has_bgad"]:
        skip.add("bgad")
    if not cfg["has_bp"]:
        skip.add("bp")
    io = {}
    for name, (shape, dt) in shapes.items():
        if name in skip:
            continue
        io[name] = nc.dram_tensor(name, shape, dt, kind="ExternalInput").ap()
    io["out_fm"] = nc.dram_tensor("out_fm", [E, T], bf,
                                  kind="ExternalOutput").ap()
    with tile.TileContext(nc) as tc:
        _build(tc, io, cfg)
    nc.compile()
    _CACHE[key] = (nc, set(io) - {"out_fm"})
    return _CACHE[key]


def kernel(**inputs):
    from concourse import bass_utils

    w, cfg = _prep_weights(inputs)
    nc, in_names = _get_program(cfg)

    tokens = np.asarray(inputs["tokens"], np.float32)
    in_maps = []
    for c in range(NCORES):
        m = {k: v for k, v in w.items() if k in in_names}
        x = tokens[c * BLOC:(c + 1) * BLOC].reshape(T, E).T
        m["x_fm"] = np.ascontiguousarray(x).astype(BF16)
        in_maps.append(m)

    res = bass_utils.run_bass_kernel_spmd(nc, in_maps, core_ids=list(range(NCORES)))
    out = np.empty((B, N, E), np.float32)
    for c in range(NCORES):
        of = res.results[c]["out_fm"]
        out[c * BLOC:(c + 1) * BLOC] = of.T.reshape(BLOC, N, E)
    return out
